# revision 9
# baseline (speedup 1.0000x reference)
"""BerHu loss kernel for Trainium2, 8-core data-parallel, fp16 inputs.

Reference computation (per sample n over its S = 1*480*640 elements):
    d  = pred - tgt
    c  = max|d| / 5
    berhu = |d|                 where |d| <= c
          = (d^2 + c^2) / (2c)  otherwise
    loss = mean_n mean_i berhu

Identity: berhu = |d| + relu(|d| - c)^2 / (2c), so
    loss_n = [ sum(ad) + (1/(2c_n)) * sum(relu(ad - c_n)^2) ] / S.

Division of labor per core (8 samples, each [128 x 2400] in SBUF):
  DVE  pass1  ABSDIFF2X (custom, hand-written 2x fp16 uops):
              ad = |p - t| (fp16 out), accum MAX -> per-partition max
  ACT         activation(Identity, accum add) over ad -> sum(ad) partials
  PE + Pool   c-chain: PE-transpose per-partition max -> [1,128] PSUM,
              gpsimd max-reduce -> m, gpsimd scale 0.2 -> c_n (saved),
              PE ones-matmul broadcast -> cb[128,1] PSUM
  DVE  pass2  RELUSQ2X (custom, 2x): accum ADD of relu(ad - c)^2 -> Q_n
Host: loss = [sum(adsum) + sum_n (0.5/c_n) * sum_p Q[p,n]] / (N*S).

The 2x uop programs implement the unshipped "T1" mechanism: per-NEFF DVE
table rows get a 2X_1PORT variant (dve_table_gen 8-aligns table_ptr), and
the instruction's byte-36[7:6] perf_max=1 lets the engine pick it when the
fp16/step-1/4B-aligned preconditions hold (silent 1x fallback otherwise).
HBM traffic halves vs fp32: 9.83 MB/core -> ~27.5 us at 358 GB/s.
"""

import os

import numpy as np

N = 64          # batch
S = 307200      # 1*480*640 elements per sample
NCORES = 8
NLOC = N // NCORES   # samples per core
P = 128              # SBUF partitions
F = S // P           # 2400 columns per sample

USE_2X = os.environ.get("BERHU_NO2X", "") != "1"

_PROG = None


def _uops_2x_absdiff():
    """2X_1PORT program: ad_lo/ad_hi = |a-b| for the packed fp16 pair,
    running MAX accumulated at stage 7 (read back via the a_flop, which is
    what DVE_READ_ACCUMULATOR2_ANT's stock program reads).

    Steady-state register plan (delay lanes):
      in: d0=a_lo d1=b_lo d2=a_hi d3=b_hi d4=ZERO
      s0: alu = a_lo - b_lo                      (d_lo)
      s1: alu = a_hi - b_hi (d_hi); d0 <- d_lo
      s2: alu = 0 - d_lo (-d_lo);   d1 <- d_hi
      s3: alu = 0 - d_hi (-d_hi);   d2 <- -d_lo
      s4: alu = max(d_lo, -d_lo)  = ad_lo; d3 <- -d_hi
      s5: alu = max(d_hi, -d_hi)  = ad_hi; d0 <- ad_lo
      s6: alu = max(ad_lo, ad_hi) = pairmax; d1 <- ad_hi
      s7: alu = max(CURR, pairmax) -> accumulator (a_flop)
      out: WR0_LO = DELAY_0 (ad_lo), WR0_HI = DELAY_1 (ad_hi)
    """
    from concourse.dve_uop import (ENABLE, AluInp, AluOp, DelayInp, InpSel,
                                   OutPath, OutSel, Trigger, UopConfig,
                                   UopDpConfig)

    A = AluInp

    def dp():
        return UopDpConfig()

    steady = UopConfig()
    steady.enable_input(InpSel.SRC_0, 1)
    steady.enable_input(InpSel.SRC_1, 2)
    steady.enable_input(InpSel.SRC_0_HI, 3)
    steady.enable_input(InpSel.SRC_1_HI, 4)
    steady.enable_input(InpSel.ZERO, 5)
    d = steady.datapath_config
    d[0] = dp().enable_alu(AluOp.SUBTRACT, A.PREV_DELAY_0, A.PREV_DELAY_1) \
        .pass_through_delay(2, 3, 4)
    d[1] = dp().enable_alu(AluOp.SUBTRACT, A.PREV_DELAY_2, A.PREV_DELAY_3) \
        .enable_delay_from_src(DelayInp.PREV_ALU_OUT, 0).pass_through_delay(4)
    d[2] = dp().enable_alu(AluOp.SUBTRACT, A.PREV_DELAY_4, A.PREV_DELAY_0) \
        .enable_delay_from_src(DelayInp.PREV_ALU_OUT, 1).pass_through_delay(0, 4)
    d[3] = dp().enable_alu(AluOp.SUBTRACT, A.PREV_DELAY_4, A.PREV_DELAY_1) \
        .enable_delay_from_src(DelayInp.PREV_ALU_OUT, 2).pass_through_delay(0, 1)
    d[4] = dp().enable_alu(AluOp.MAX, A.PREV_DELAY_0, A.PREV_DELAY_2) \
        .enable_delay_from_src(DelayInp.PREV_ALU_OUT, 3).pass_through_delay(1)
    d[5] = dp().enable_alu(AluOp.MAX, A.PREV_DELAY_1, A.PREV_DELAY_3) \
        .enable_delay_from_src(DelayInp.PREV_ALU_OUT, 0)
    d[6] = dp().enable_alu(AluOp.MAX, A.PREV_DELAY_0, A.PREV_ALU_OUT) \
        .enable_delay_from_src(DelayInp.PREV_ALU_OUT, 1).pass_through_delay(0)
    d[7] = dp().enable_alu(AluOp.MAX, A.CURR_ALU_OUT, A.PREV_ALU_OUT) \
        .pass_through_delay(0, 1)
    d[7].alu_out_a_enable = ENABLE
    steady.enable_output(OutSel.DELAY_0, OutPath.WR0_LO)
    steady.enable_output(OutSel.DELAY_1, OutPath.WR0_HI)
    steady.require_inp0 = ENABLE
    steady.require_inp1 = ENABLE
    steady.trigger = (Trigger.SRC_TENSOR_DONE, Trigger.NONE, Trigger.NONE)
    steady.next_uop = (0, 0, 0)

    # Init: one bubble (no src consumed) rides MAX_NEG down lane d4 and
    # seeds stage 7's accumulator flops.
    init = UopConfig()
    init.enable_input(InpSel.MAX_NEG, 5)
    di = init.datapath_config
    for k in range(7):
        di[k] = dp().pass_through_delay(4)
    di[7] = dp().enable_alu(AluOp.BYPASS, A.PREV_DELAY_4, A.PREV_DELAY_4)
    di[7].alu_out_a_enable = ENABLE
    init.repeat_count = 1
    init.trigger = (Trigger.COUNT, Trigger.NONE, Trigger.NONE)
    init.next_uop = (1, 0, 0)
    return [init, steady]


def _uops_2x_relusq():
    """2X_1PORT program: q = relu(x - c)^2 on the packed fp16 pair, running
    SUM accumulated at stage 7.

      in: d0=x_lo d1=x_hi d2=CONST_0(c) d3=ZERO
      s0: alu = x_lo - c                 (y_lo)
      s1: alu = x_hi - c (y_hi); d0 <- y_lo
      s2: alu = max(y_lo, 0)  (r_lo); d1 <- y_hi
      s3: alu = max(y_hi, 0)  (r_hi); d0 <- r_lo
      s4: alu = r_lo * r_lo   (q_lo); d1 <- r_hi
      s5: alu = r_hi * r_hi   (q_hi); d0 <- q_lo
      s6: alu = q_lo + q_hi (pairsum); d1 <- q_hi
      s7: alu = CURR + pairsum -> accumulator (a_flop)
      out: WR0_LO = DELAY_0 (q_lo), WR0_HI = DELAY_1 (q_hi)
    """
    from concourse.dve_uop import (ENABLE, AluInp, AluOp, DelayInp, InpSel,
                                   OutPath, OutSel, Trigger, UopConfig,
                                   UopDpConfig)

    A = AluInp

    def dp():
        return UopDpConfig()

    steady = UopConfig()
    steady.enable_input(InpSel.SRC_0, 1)
    steady.enable_input(InpSel.SRC_0_HI, 2)
    steady.enable_input(InpSel.CONST_0, 3)
    steady.enable_input(InpSel.ZERO, 4)
    d = steady.datapath_config
    d[0] = dp().enable_alu(AluOp.SUBTRACT, A.PREV_DELAY_0, A.PREV_DELAY_2) \
        .pass_through_delay(1, 2, 3)
    d[1] = dp().enable_alu(AluOp.SUBTRACT, A.PREV_DELAY_1, A.PREV_DELAY_2) \
        .enable_delay_from_src(DelayInp.PREV_ALU_OUT, 0).pass_through_delay(3)
    d[2] = dp().enable_alu(AluOp.MAX, A.PREV_DELAY_0, A.PREV_DELAY_3) \
        .enable_delay_from_src(DelayInp.PREV_ALU_OUT, 1).pass_through_delay(3)
    d[3] = dp().enable_alu(AluOp.MAX, A.PREV_DELAY_1, A.PREV_DELAY_3) \
        .enable_delay_from_src(DelayInp.PREV_ALU_OUT, 0)
    d[4] = dp().enable_alu(AluOp.MULTIPLY, A.PREV_DELAY_0, A.PREV_DELAY_0) \
        .enable_delay_from_src(DelayInp.PREV_ALU_OUT, 1)
    d[5] = dp().enable_alu(AluOp.MULTIPLY, A.PREV_DELAY_1, A.PREV_DELAY_1) \
        .enable_delay_from_src(DelayInp.PREV_ALU_OUT, 0)
    d[6] = dp().enable_alu(AluOp.ADD, A.PREV_DELAY_0, A.PREV_ALU_OUT) \
        .enable_delay_from_src(DelayInp.PREV_ALU_OUT, 1).pass_through_delay(0)
    d[7] = dp().enable_alu(AluOp.ADD, A.CURR_ALU_OUT, A.PREV_ALU_OUT) \
        .pass_through_delay(0, 1)
    d[7].alu_out_a_enable = ENABLE
    steady.enable_output(OutSel.DELAY_0, OutPath.WR0_LO)
    steady.enable_output(OutSel.DELAY_1, OutPath.WR0_HI)
    steady.require_inp0 = ENABLE
    steady.trigger = (Trigger.SRC_TENSOR_DONE, Trigger.NONE, Trigger.NONE)
    steady.next_uop = (0, 0, 0)

    init = UopConfig()
    init.enable_input(InpSel.ZERO, 4)
    di = init.datapath_config
    for k in range(7):
        di[k] = dp().pass_through_delay(3)
    di[7] = dp().enable_alu(AluOp.BYPASS, A.PREV_DELAY_3, A.PREV_DELAY_3)
    di[7].alu_out_a_enable = ENABLE
    init.repeat_count = 1
    init.trigger = (Trigger.COUNT, Trigger.NONE, Trigger.NONE)
    init.next_uop = (1, 0, 0)
    return [init, steady]


def _register_ops():
    import concourse.dve_ops as dve_ops
    from concourse.dve_ops import OPS, DveOp, has_src1
    from concourse.dve_spec import (C0, AluOp, Spec, Src0, Src1, Zero, lower,
                                    maxx, relu, sq)
    from concourse.dve_uop import DveOpSpec

    class DveOp2x(DveOp):
        """DveOp whose compiled table rows carry a hand-written 2X_1PORT
        variant (the unshipped T1 mechanism; 1x program still from
        lower())."""

        def compile(self, ver):
            cache = getattr(self, "_c2x", None)
            if cache is not None and cache[0] == ver:
                return cache[1]
            uops_2x = self._uops_2x_fn() if (USE_2X and ver == "v3") else None
            result = DveOpSpec(
                name=self.name,
                opcode=dve_ops.get_dve_sub_opcode(self.name),
                uops=lower(self.spec, ver=ver),
                uops_2x=uops_2x,
                perf_max=1 if uops_2x is not None else 0,
                rd1_en=has_src1(self.spec),
            )
            self._c2x = (ver, result)
            return result

    def add_op(name, spec, uops_2x_fn):
        for o in OPS:
            if o.name == name:
                return o
        op = DveOp2x(name, spec, subdim=False, uops_sha={})
        op._uops_2x_fn = uops_2x_fn
        OPS.append(op)
        dve_ops.CUSTOM_DVE_SPECS[name] = spec
        dve_ops._SUB_OPCODE_FOR_NAME[name] = (
            dve_ops._CUSTOM_DVE_ROW_BASE + len(OPS) - 1)
        assert dve_ops._SUB_OPCODE_FOR_NAME[name] < 0x20
        return op

    def _absdiff_ref(in0, in1, c0, c1, c2):
        x = in0.astype(np.float32).reshape(in0.shape[0], -1)
        y = np.asarray(in1, np.float32).reshape(in0.shape[0], -1)
        out = np.abs(x - y).astype(np.float32)
        return out, out.max(axis=-1)

    def _relusq_ref(in0, in1, c0, c1, c2):
        x = in0.astype(np.float32).reshape(in0.shape[0], -1)
        r = np.maximum(x - c0, 0.0).astype(np.float32)
        out = (r * r).astype(np.float32)
        return out, out.sum(axis=-1, dtype=np.float32)

    d = Src0 - Src1
    absdiff = add_op(
        "ANT_BERHU_ABSDIFF2X",
        Spec(body=maxx(d, Zero - d), accum=AluOp.MAX, reference=_absdiff_ref),
        _uops_2x_absdiff,
    )
    relusq = add_op(
        "ANT_BERHU_RELUSQ2X",
        Spec(body=sq(relu(Src0 - C0)), accum=AluOp.ADD, reference=_relusq_ref),
        _uops_2x_relusq,
    )
    return absdiff, relusq


def _set_perf(binst):
    """Mark the emitted InstCustomDveAnt as 2x-capable (byte-36[7:6])."""
    if not USE_2X:
        return
    ins = getattr(binst, "ins", binst)
    ins.perf_max = 1


def _build(repeat=1, loop_n=None):
    """Build the per-core program. `repeat` > 1 replays the whole 8-sample
    body that many times inside one NEFF (unrolled); `loop_n` wraps the body
    in a device-side For_i loop (benchmarking only)."""
    from contextlib import ExitStack

    import concourse.bacc as bacc
    import concourse.tile as tile
    from concourse import mybir

    absdiff_op, relusq_op = _register_ops()

    f32 = mybir.dt.float32
    f16 = mybir.dt.float16
    Alu = mybir.AluOpType

    nc = bacc.Bacc("TRN2", target_bir_lowering=False, debug=False,
                   num_devices=NCORES)
    p_d = nc.dram_tensor("p", [NLOC * P, F], f16, kind="ExternalInput").ap()
    t_d = nc.dram_tensor("t", [NLOC * P, F], f16, kind="ExternalInput").ap()
    # adsumq: cols 0..NLOC-1 = per-partition sum(ad), cols NLOC.. = Q_n
    bh_d = nc.dram_tensor("bh", [P, 2 * NLOC], f32, kind="ExternalOutput").ap()
    # msc row 0: c_n per local sample
    ms_d = nc.dram_tensor("ms", [1, NLOC], f32, kind="ExternalOutput").ap()

    with tile.TileContext(nc) as tc, ExitStack() as ctx:
        io = ctx.enter_context(tc.tile_pool(name="io", bufs=2))
        work = ctx.enter_context(tc.tile_pool(name="work", bufs=3))
        work2 = ctx.enter_context(tc.tile_pool(name="work2", bufs=2))
        small = ctx.enter_context(tc.tile_pool(name="small", bufs=3))
        stats = ctx.enter_context(tc.tile_pool(name="stats", bufs=1))

        adsumq = stats.tile([P, 2 * NLOC], f32, tag="adsumq")
        msc = stats.tile([1, NLOC], f32, tag="msc")
        junkA = stats.tile([P, F], f16, tag="junkA")
        ones_t = stats.tile([1, P], f32, tag="ones")
        nc.vector.memset(ones_t[:], 1.0)
        total = NLOC * repeat

        quad = {}
        # Sample-group DMA schedule: a 1-sample group first (short pipeline
        # fill), 2-sample groups in the middle (fewer DMAs), 1-sample last
        # (short tail after the final transfer lands).
        GROUPS = [[0], [1, 2], [3, 4], [5, 6], [7]]
        GROUP_OF = {}
        for g in GROUPS:
            for s in g:
                GROUP_OF[s] = g

        def load_group(g):
            n, cnt = g[0], len(g)
            rows = slice(n * P, (n + cnt) * P)
            pt = io.tile([P, 2 * F], f16, tag="p")
            tt = io.tile([P, 2 * F], f16, tag="t")
            src_p = p_d[rows, :].rearrange("(s p) f -> p s f", s=cnt)
            src_t = t_d[rows, :].rearrange("(s p) f -> p s f", s=cnt)
            nc.sync.dma_start(
                out=pt[:, :cnt * F].rearrange("p (s f) -> p s f", s=cnt),
                in_=src_p)
            nc.sync.dma_start(
                out=tt[:, :cnt * F].rearrange("p (s f) -> p s f", s=cnt),
                in_=src_t)
            quad["p"], quad["t"] = pt, tt

        def pass1(i):
            n = i % NLOC
            g = GROUP_OF[n]
            if n == g[0]:
                load_group(g)
            k = n - g[0]
            cols = slice(k * F, (k + 1) * F)
            ad = work.tile([P, F], f16, tag="ad")
            mxn = small.tile([P, 1], f32, tag="mxn")
            r = nc.vector._custom_dve(absdiff_op, out=ad[:],
                                      in0=quad["p"][:, cols],
                                      in1=quad["t"][:, cols],
                                      accum_out=mxn[:])
            _set_perf(r)
            return {"ad": ad, "mxn": mxn}

        def actsum(i, st):
            # ACT: sum(ad) per partition -> adsumq[:, n]
            n = i % NLOC
            nc.scalar.activation(
                out=junkA[:], in_=st["ad"][:],
                func=mybir.ActivationFunctionType.Identity,
                accum_out=adsumq[:, n:n + 1])

        def chain(i, st):
            # gpsimd all-reduce max across partitions (m lands on every
            # partition), scale by 0.2 -> per-partition c for pass2; one
            # copy of c into msc row 0 for the host.
            from concourse import bass_isa
            n = i % NLOC
            cball = small.tile([P, 1], f32, tag="cball")
            nc.gpsimd.partition_all_reduce(cball[:], st["mxn"][:],
                                           channels=P,
                                           reduce_op=bass_isa.ReduceOp.max)
            cbs = small.tile([P, 1], f32, tag="cbs")
            nc.gpsimd.tensor_scalar_mul(out=cbs[:], in0=cball[:], scalar1=0.2)
            nc.gpsimd.tensor_scalar_mul(out=msc[0:1, n:n + 1],
                                        in0=cball[0:1, 0:1], scalar1=0.2)
            st["cb"] = cbs

        def pass2(i, st):
            # Q_n[p] = sum(relu(ad - c)^2); host scales by 1/(2c)
            n = i % NLOC
            junk = work2.tile([P, F], f16, tag="junk")
            r = nc.vector._custom_dve(relusq_op, out=junk[:], in0=st["ad"][:],
                                      s0=st["cb"][:, 0:1],
                                      accum_out=adsumq[:, NLOC + n:NLOC + n + 1])
            _set_perf(r)

        # 2-deep software pipeline: pass1(i) | chain(i-1) | pass2(i-2) keeps
        # the DVE stream free of waits on the c-derivation chain.
        def body():
            hist = {}
            for i in range(total):
                hist[i] = pass1(i)
                actsum(i, hist[i])
                if i - 1 >= 0:
                    chain(i - 1, hist[i - 1])
                if i - 2 >= 0:
                    pass2(i - 2, hist.pop(i - 2))
            for i in (total - 2, total - 1):
                if i >= 0:
                    if "cb" not in hist[i]:
                        chain(i, hist[i])
                    pass2(i, hist.pop(i))

        if loop_n is not None:
            with tc.For_i(0, loop_n, 1):
                body()
        else:
            body()

        nc.sync.dma_start(out=bh_d[:], in_=adsumq[:])
        nc.sync.dma_start(out=ms_d[:], in_=msc[:])

    nc.compile()
    return nc


def _get_prog():
    global _PROG
    if _PROG is None:
        _PROG = _build()
    return _PROG


def _combine(results):
    total = 0.0
    for r in results:
        bh = r["bh"].astype(np.float64)       # [P, 2*NLOC]
        cs = r["ms"].astype(np.float64)[0]    # [NLOC] = c_n
        adsum = bh[:, :NLOC].sum()
        q = bh[:, NLOC:].sum(axis=0)          # [NLOC]
        i2c = np.where(cs > 1e-20, 0.5 / np.maximum(cs, 1e-20), 0.0)
        total += adsum + (q * i2c).sum()
    return np.float32(total / (N * S))


def bench_inputs(p_all, t_all):
    """Per-core input arrays for the bench harness."""
    return {
        "p": p_all.astype(np.float16).reshape(NCORES, NLOC * P, F),
        "t": t_all.astype(np.float16).reshape(NCORES, NLOC * P, F),
    }


def kernel(predictions, targets):
    from concourse.bass_utils import run_bass_kernel_spmd

    nc = _get_prog()
    p = np.ascontiguousarray(
        np.asarray(predictions).astype(np.float16).reshape(
            NCORES, NLOC * P, F))
    t = np.ascontiguousarray(
        np.asarray(targets).astype(np.float16).reshape(
            NCORES, NLOC * P, F))
    in_maps = [{"p": p[k], "t": t[k]} for k in range(NCORES)]
    res = run_bass_kernel_spmd(nc, in_maps, list(range(NCORES)))
    return _combine(res.results)
